# revision 12
# baseline (speedup 1.0000x reference)
"""Multi-head attention (B=2, S=4096, D=1024, H=16, HD=64) on 8 trn2 cores.

Sharding: core c -> batch b = c//4, head-group g = c%4 (4 heads per core).
Each core: Q/K/V projections for its heads on its batch, attention, and the
partial output ctx @ Wo[rows of its heads]. Host sums the 4 partials per
batch and adds bo.

v2 design (vs f32r baseline at 760us):
  - All matmul operands bf16. X is pre-transposed and pre-converted to bf16
    on the host, so the 256 on-chip PE transposes of the baseline are gone.
  - Scores st^T[k, 1024] per step: two 64-contraction matmuls on disjoint
    PE row groups (explicit tile_position + skip_group_check + separate
    PSUM banks) so the pair runs concurrently.
  - exp is the scalar-engine bottleneck (512 x 1008ns in the baseline), so
    steps alternate between ACT exp and a DVE Schraudolph approximation
    (i16 = st*A/8 + B bitcast to bf16), ratio ACT_PAT, amortizing the
    softmax across both engines. Softmax normalization cancels the ~2%
    Schraudolph noise.
  - PV: one 128-contraction matmul per head into acc[65,512] (V has a ones
    column -> row 64 accumulates the softmax denominator).
  - Normalization fused at k==KT-1: reciprocal of denom row, PE outer-
    product broadcast to [64,512], DVE multiply straight into the stacked
    bf16 ctx tile [128 = 2 heads x 64 dims, q] that feeds Wo as lhsT.
  - ph3: per 128-token tile, 2 accumulating bf16 matmuls (K=128, N=1024).
"""

import os
from contextlib import ExitStack

import numpy as np
import ml_dtypes

os.environ.setdefault("MYCRO_LOCAL_CACHE", "1")

import concourse.bass as bass
import concourse.tile as tile
from concourse import bacc, mybir
from concourse.bass_utils import run_bass_kernel_spmd

F32 = mybir.dt.float32
F32R = mybir.dt.float32r
BF16 = mybir.dt.bfloat16
I16 = mybir.dt.int16
AF = mybir.ActivationFunctionType
ALU = mybir.AluOpType

S = 4096          # sequence length
D = 1024          # model dim
HC = 4            # heads per core
HD = 64           # head dim
DC = HC * HD      # 256 per-core projection width
NP = HC // 2      # head pairs per core
KT = S // 128     # 32 k-tiles
QC = S // 512     # 8 q-chunks of 512
SCALE = 1.0 / 8.0

# Schraudolph exp approximation in bf16: i16 = round(x * 128/ln2 + 127*128 + C)
# with the 1/8 score scale folded into A. C tuned so the relative error is
# zero-mean (the DVE converts with truncation; +0.5 folded in).
SCH_A = (128.0 / np.log(2.0)) / 8.0
SCH_B = 127.0 * 128.0 - 11.3 + 0.5

# Which of every 8 ph2 steps use the ACT engine for exp (rest use DVE).
ACT_PAT = (0, 1, 3, 4, 6)


def _emit(ctx: ExitStack, tc: tile.TileContext, ins: dict, out: bass.AP):
    nc = tc.nc
    XT, Wq, bq, Wk, bk, Wv, bv, Wo = (
        ins["XT"], ins["Wq"], ins["bq"], ins["Wk"], ins["bk"], ins["Wv"],
        ins["bv"], ins["Wo"],
    )

    const = ctx.enter_context(tc.tile_pool(name="const", bufs=1))
    wq_sb = const.tile([128, 8, DC], BF16, tag="wq")
    wk_sb = const.tile([128, 8, DC], BF16, tag="wk")
    wv_sb = const.tile([128, 8, DC], BF16, tag="wv")
    wo_sb = const.tile([128, 2, D], BF16, tag="wo")
    for dst, src, nch in ((wq_sb, Wq, 8), (wk_sb, Wk, 8), (wv_sb, Wv, 8),
                          (wo_sb, Wo, 2)):
        nc.sync.dma_start(dst[:], src.rearrange("(c p) d -> p c d", p=128))
    bq_sb = const.tile([128, 2], F32, tag="bq")
    bk_sb = const.tile([128, 2], F32, tag="bk")
    nc.sync.dma_start(bq_sb[:], bq.rearrange("(c p) -> p c", p=128))
    nc.sync.dma_start(bk_sb[:], bk.rearrange("(c p) -> p c", p=128))
    bv_bc = const.tile([128, DC], F32, tag="bv")
    nc.sync.dma_start(bv_bc[:], bv.unsqueeze(0).to_broadcast([128, DC]))
    ones1 = const.tile([1, 64], BF16, tag="ones1")
    nc.vector.memset(ones1[:], 1.0)
    ones4 = const.tile([128, HC], BF16, tag="ones4")
    nc.vector.memset(ones4[:], 1.0)

    acts = ctx.enter_context(tc.tile_pool(name="acts", bufs=1))
    QT = [acts.tile([128, S], BF16, tag=f"qt{p}", name=f"qt{p}") for p in range(NP)]
    KT_ = [acts.tile([128, S], BF16, tag=f"kt{p}", name=f"ktile{p}") for p in range(NP)]
    VPA = acts.tile([128, KT, HC, 65], BF16, tag="vpa", name="vpa")
    # normalized ctx^T, stacked [2 heads x 64 dims, q] per (p, qc): Wo lhsT
    CTX = acts.tile([128, NP, QC, 512], BF16, tag="ctx", name="ctx")

    # ---------------- Phase 1: projections ----------------
    with nc.named_scope("ph1"), \
         tc.tile_pool(name="xt", bufs=1) as xtp, \
         tc.tile_pool(name="ps1", bufs=2, space="PSUM") as ps1, \
         tc.tile_pool(name="ps2", bufs=2, space="PSUM") as ps2:
        xt = xtp.tile([128, 8, S], BF16, tag="xt")
        for dc in range(8):
            nc.sync.dma_start(xt[:, dc, :], XT[dc * 128:(dc + 1) * 128, :])
        # K^T and Q^T: [dq(128 = head pair), tok], bf16, bias folded via DVE
        for p in range(NP):
            for w_sb, b_sb, dstT in ((wk_sb, bk_sb, KT_[p]), (wq_sb, bq_sb, QT[p])):
                for qch in range(8):
                    pq = ps1.tile([128, 512], F32, tag="pq")
                    for dc in range(8):
                        nc.tensor.matmul(
                            pq[:], w_sb[:, dc, p * 128:(p + 1) * 128],
                            xt[:, dc, qch * 512:(qch + 1) * 512],
                            start=(dc == 0), stop=(dc == 7))
                    nc.vector.tensor_scalar_add(
                        dstT[:, qch * 512:(qch + 1) * 512], pq[:],
                        b_sb[:, p:p + 1])
        # V token-major with bias, interleaved into VPA
        for tt in range(KT):
            pv = ps2.tile([128, DC], F32, tag="pv")
            for dc in range(8):
                nc.tensor.matmul(
                    pv[:], xt[:, dc, tt * 128:(tt + 1) * 128],
                    wv_sb[:, dc, :], start=(dc == 0), stop=(dc == 7))
            nc.vector.scalar_tensor_tensor(
                VPA[:, tt, :, 0:64], pv[:].rearrange("p (h w) -> p h w", h=HC),
                1.0, bv_bc[:].rearrange("p (h w) -> p h w", h=HC),
                ALU.bypass, ALU.add)
            nc.vector.tensor_copy(VPA[:, tt, :, 64:65], ones4[:].unsqueeze(2))

    # ---------------- Phase 2: attention ----------------
    with nc.named_scope("ph2"), \
         tc.tile_pool(name="sps", bufs=2, space="PSUM") as sps, \
         tc.tile_pool(name="pvs", bufs=3, space="PSUM") as pvs, \
         tc.tile_pool(name="rbp", bufs=1, space="PSUM") as rbp, \
         tc.tile_pool(name="et", bufs=4) as etp, \
         tc.tile_pool(name="rcp", bufs=4) as rcpp:
        seq = [(p, qc, k) for p in range(NP) for qc in range(QC)
               for k in range(KT)]
        accs = {}
        ets = {}

        def s_step(i):
            p, qc, k = seq[i]
            qs = slice(qc * 512, (qc + 1) * 512)
            ks = slice(k * 128, (k + 1) * 128)
            st = sps.tile([128, 1024], F32, tag="st", name=f"st{p}_{qc}_{k}")
            nc.tensor.matmul(st[:, 0:512], KT_[p][0:64, ks], QT[p][0:64, qs],
                             start=True, stop=True, skip_group_check=True)
            nc.tensor.matmul(st[:, 512:1024], KT_[p][64:128, ks],
                             QT[p][64:128, qs], start=True, stop=True,
                             skip_group_check=True)
            et = etp.tile([128, 1024], BF16, tag="et", name=f"et{p}_{qc}_{k}")
            if (i % 8) in ACT_PAT:
                nc.scalar.activation(et[:], st[:], AF.Exp, bias=0.0,
                                     scale=SCALE)
            else:
                nc.vector.tensor_scalar(et[:].bitcast(I16), st[:], SCH_A,
                                        SCH_B, ALU.mult, ALU.add)
            ets[i] = et

        LOOKAHEAD = 2
        for i in range(LOOKAHEAD):
            s_step(i)
        for i, (p, qc, k) in enumerate(seq):
            if i + LOOKAHEAD < len(seq):
                s_step(i + LOOKAHEAD)
            if k == 0:
                accs[(p, qc)] = [
                    pvs.tile([65, 512], F32, tag="acc", name=f"acc{p}_{qc}_{j}")
                    for j in range(2)]
            accA, accB = accs[(p, qc)]
            et = ets.pop(i)
            nc.tensor.matmul(accA[:], VPA[:, k, 2 * p, :], et[:, 0:512],
                             start=(k == 0), stop=(k == KT - 1),
                             skip_group_check=True)
            nc.tensor.matmul(accB[:], VPA[:, k, 2 * p + 1, :], et[:, 512:1024],
                             start=(k == 0), stop=(k == KT - 1),
                             skip_group_check=True)
            if k == KT - 1:
                rcpb = rbp.tile([128, 512], F32, tag="rcpb",
                                name=f"rcpb{p}_{qc}")
                with nc.allow_low_precision(reason="f32r keeps fp32 range"):
                    for j, acc in enumerate((accA, accB)):
                        rcp = rcpp.tile([1, 512], BF16, tag="rcp",
                                        name=f"rcp{p}_{qc}_{j}")
                        nc.vector.reciprocal(rcp[:], acc[64:65, :])
                        nc.tensor.matmul(rcpb[j * 64:(j + 1) * 64, :],
                                         ones1[:], rcp[:],
                                         start=True, stop=True,
                                         skip_group_check=True)
                # DVE may read only one PSUM operand per op: stage the
                # broadcast reciprocals in SBUF, then multiply acc (PSUM)
                # by them.
                rcps = rcpp.tile([128, 512], F32, tag="rcps",
                                 name=f"rcps{p}_{qc}")
                nc.vector.tensor_copy(rcps[:], rcpb[:])
                for j, acc in enumerate((accA, accB)):
                    nc.vector.tensor_tensor(
                        CTX[j * 64:(j + 1) * 64, p, qc, :], acc[0:64, :],
                        rcps[j * 64:(j + 1) * 64, :], ALU.mult)
                del accs[(p, qc)]

    # ---------------- Phase 3: Wo ----------------
    with nc.named_scope("ph3"), \
         tc.tile_pool(name="po", bufs=2, space="PSUM") as pop, \
         tc.tile_pool(name="osb", bufs=3) as osbp:
        for t in range(S // 128):
            qc, tt = divmod(t, 4)
            cs = slice(tt * 128, (tt + 1) * 128)
            po = pop.tile([128, D], F32, tag="po", name=f"po{t}")
            for n2 in range(2):
                for p in range(NP):
                    nc.tensor.matmul(
                        po[:, n2 * 512:(n2 + 1) * 512], CTX[:, p, qc, cs],
                        wo_sb[:, p, n2 * 512:(n2 + 1) * 512],
                        start=(p == 0), stop=(p == NP - 1))
            ot = osbp.tile([128, D], F32, tag="ot", name=f"ot{t}")
            nc.scalar.copy(ot[:, 0:512], po[:, 0:512])
            nc.vector.tensor_copy(ot[:, 512:1024], po[:, 512:1024])
            nc.sync.dma_start(out[t * 128:(t + 1) * 128, :], ot[:])


_CACHE = {}


def _build():
    if "nc" in _CACHE:
        return _CACHE["nc"]
    nc = bacc.Bacc("TRN2", target_bir_lowering=False, debug=False)
    ins = {
        "XT": nc.dram_tensor("XT", [D, S], BF16, kind="ExternalInput").ap(),
        "Wq": nc.dram_tensor("Wq", [D, DC], BF16, kind="ExternalInput").ap(),
        "bq": nc.dram_tensor("bq", [DC], F32, kind="ExternalInput").ap(),
        "Wk": nc.dram_tensor("Wk", [D, DC], BF16, kind="ExternalInput").ap(),
        "bk": nc.dram_tensor("bk", [DC], F32, kind="ExternalInput").ap(),
        "Wv": nc.dram_tensor("Wv", [D, DC], BF16, kind="ExternalInput").ap(),
        "bv": nc.dram_tensor("bv", [DC], F32, kind="ExternalInput").ap(),
        "Wo": nc.dram_tensor("Wo", [DC, D], BF16, kind="ExternalInput").ap(),
    }
    outp = nc.dram_tensor("out", [S, D], F32, kind="ExternalOutput").ap()
    with tile.TileContext(nc) as tcx:
        with ExitStack() as ctx:
            _emit(ctx, tcx, ins, outp)
    nc.compile()
    _CACHE["nc"] = nc
    return nc


def core_inputs(X, Wq, bq, Wk, bk, Wv, bv, Wo, core):
    b, g = divmod(core, 4)
    cs = slice(g * DC, (g + 1) * DC)
    bf = ml_dtypes.bfloat16
    return {
        "XT": np.ascontiguousarray(np.asarray(X[b]).T).astype(bf),
        "Wq": np.ascontiguousarray(Wq[:, cs]).astype(bf),
        "bq": np.ascontiguousarray(bq[cs], dtype=np.float32),
        "Wk": np.ascontiguousarray(Wk[:, cs]).astype(bf),
        "bk": np.ascontiguousarray(bk[cs], dtype=np.float32),
        "Wv": np.ascontiguousarray(Wv[:, cs]).astype(bf),
        "bv": np.ascontiguousarray(bv[cs], dtype=np.float32),
        "Wo": np.ascontiguousarray(Wo[cs, :]).astype(bf),
    }


def kernel(X, Wq, bq, Wk, bk, Wv, bv, Wo, bo, _trace=False):
    nc = _build()
    in_maps = [core_inputs(X, Wq, bq, Wk, bk, Wv, bv, Wo, c) for c in range(8)]
    res = run_bass_kernel_spmd(nc, in_maps, list(range(8)), trace=_trace)
    parts = [res.results[c]["out"] for c in range(8)]
    bo32 = np.asarray(bo, dtype=np.float32)
    full = np.stack([
        parts[0] + parts[1] + parts[2] + parts[3] + bo32,
        parts[4] + parts[5] + parts[6] + parts[7] + bo32,
    ]).astype(np.float32)
    if _trace:
        return full, res
    return full


# revision 19
# speedup vs baseline: 1.0061x; 1.0061x over previous
"""Multi-head attention (B=2, S=4096, D=1024, H=16, HD=64) on 8 trn2 cores.

Sharding: core c -> batch b = c//4, head-group g = c%4 (4 heads per core).
Each core: Q/K/V projections for its heads on its batch, attention, and the
partial output ctx @ Wo[rows of its heads]. Host sums the 4 partials per
batch and adds bo.

v2 design (vs f32r baseline at 760us):
  - All matmul operands bf16. X is pre-transposed and pre-converted to bf16
    on the host, so the 256 on-chip PE transposes of the baseline are gone.
  - Scores st^T[k, 1024] per step: two 64-contraction matmuls on disjoint
    PE row groups (explicit tile_position + skip_group_check + separate
    PSUM banks) so the pair runs concurrently.
  - exp is the scalar-engine bottleneck (512 x 1008ns in the baseline), so
    steps alternate between ACT exp and a DVE Schraudolph approximation
    (i16 = st*A/8 + B bitcast to bf16), ratio ACT_PAT, amortizing the
    softmax across both engines. Softmax normalization cancels the ~2%
    Schraudolph noise.
  - PV: one 128-contraction matmul per head into acc[65,512] (V has a ones
    column -> row 64 accumulates the softmax denominator).
  - Normalization fused at k==KT-1: reciprocal of denom row, PE outer-
    product broadcast to [64,512], DVE multiply straight into the stacked
    bf16 ctx tile [128 = 2 heads x 64 dims, q] that feeds Wo as lhsT.
  - ph3: per 128-token tile, 2 accumulating bf16 matmuls (K=128, N=1024).
"""

import os
from contextlib import ExitStack

import numpy as np
import ml_dtypes

os.environ.setdefault("MYCRO_LOCAL_CACHE", "1")

import concourse.bass as bass
import concourse.tile as tile
from concourse import bacc, mybir
from concourse.bass_utils import run_bass_kernel_spmd

F32 = mybir.dt.float32
F32R = mybir.dt.float32r
BF16 = mybir.dt.bfloat16
I16 = mybir.dt.int16
AF = mybir.ActivationFunctionType
ALU = mybir.AluOpType

S = 4096          # sequence length
D = 1024          # model dim
HC = 4            # heads per core
HD = 64           # head dim
DC = HC * HD      # 256 per-core projection width
NP = HC // 2      # head pairs per core
KT = S // 128     # 32 k-tiles
QC = S // 512     # 8 q-chunks of 512
SCALE = 1.0 / 8.0

# Schraudolph exp approximation in bf16: i16 = round(x * 128/ln2 + 127*128 + C)
# with the 1/8 score scale folded into A. C tuned so the relative error is
# zero-mean (the DVE converts with truncation; +0.5 folded in).
SCH_A = (128.0 / np.log(2.0)) / 8.0
SCH_B = 127.0 * 128.0 - 11.3 + 0.5


def _emit(ctx: ExitStack, tc: tile.TileContext, ins: dict, out: bass.AP):
    nc = tc.nc
    XT, Wq, bq, Wk, bk, Wv, bv, Wo = (
        ins["XT"], ins["Wq"], ins["bq"], ins["Wk"], ins["bk"], ins["Wv"],
        ins["bv"], ins["Wo"],
    )

    const = ctx.enter_context(tc.tile_pool(name="const", bufs=1))
    wq_sb = const.tile([128, 8, DC], BF16, tag="wq")
    wk_sb = const.tile([128, 8, DC], BF16, tag="wk")
    wv_sb = const.tile([128, 8, DC], BF16, tag="wv")
    wo_sb = const.tile([128, 2, D], BF16, tag="wo")
    for dst, src, nch in ((wq_sb, Wq, 8), (wk_sb, Wk, 8), (wv_sb, Wv, 8),
                          (wo_sb, Wo, 2)):
        nc.sync.dma_start(dst[:], src.rearrange("(c p) d -> p c d", p=128))
    bq_sb = const.tile([128, 2], F32, tag="bq")
    bk_sb = const.tile([128, 2], F32, tag="bk")
    nc.sync.dma_start(bq_sb[:], bq.rearrange("(c p) -> p c", p=128))
    nc.sync.dma_start(bk_sb[:], bk.rearrange("(c p) -> p c", p=128))
    bv_bc = const.tile([128, DC], F32, tag="bv")
    nc.sync.dma_start(bv_bc[:], bv.unsqueeze(0).to_broadcast([128, DC]))
    ones1 = const.tile([1, 64], BF16, tag="ones1")
    nc.vector.memset(ones1[:], 1.0)
    ones4 = const.tile([128, HC], BF16, tag="ones4")
    nc.vector.memset(ones4[:], 1.0)

    acts = ctx.enter_context(tc.tile_pool(name="acts", bufs=1))
    QT = [acts.tile([128, S], BF16, tag=f"qt{p}", name=f"qt{p}") for p in range(NP)]
    KT_ = [acts.tile([128, S], BF16, tag=f"kt{p}", name=f"ktile{p}") for p in range(NP)]
    VPA = acts.tile([128, KT, HC, 65], BF16, tag="vpa", name="vpa")
    # normalized ctx^T, stacked [2 heads x 64 dims, q] per (p, qc): Wo lhsT
    CTX = acts.tile([128, NP, QC, 512], BF16, tag="ctx", name="ctx")

    # ---------------- Phase 1: projections ----------------
    with nc.named_scope("ph1"), \
         tc.tile_pool(name="xt", bufs=1) as xtp, \
         tc.tile_pool(name="ps1", bufs=2, space="PSUM") as ps1, \
         tc.tile_pool(name="ps2", bufs=2, space="PSUM") as ps2:
        xt = xtp.tile([128, 8, S], BF16, tag="xt")
        for dc in range(8):
            nc.sync.dma_start(xt[:, dc, :], XT[dc * 128:(dc + 1) * 128, :])
        # K^T and Q^T: [dq(128 = head pair), tok], bf16, bias folded via DVE
        for p in range(NP):
            for w_sb, b_sb, dstT in ((wk_sb, bk_sb, KT_[p]), (wq_sb, bq_sb, QT[p])):
                for qch in range(8):
                    pq = ps1.tile([128, 512], F32, tag="pq")
                    for dc in range(8):
                        nc.tensor.matmul(
                            pq[:], w_sb[:, dc, p * 128:(p + 1) * 128],
                            xt[:, dc, qch * 512:(qch + 1) * 512],
                            start=(dc == 0), stop=(dc == 7))
                    nc.vector.tensor_scalar_add(
                        dstT[:, qch * 512:(qch + 1) * 512], pq[:],
                        b_sb[:, p:p + 1])
        # V token-major with bias, interleaved into VPA
        for tt in range(KT):
            pv = ps2.tile([128, DC], F32, tag="pv")
            for dc in range(8):
                nc.tensor.matmul(
                    pv[:], xt[:, dc, tt * 128:(tt + 1) * 128],
                    wv_sb[:, dc, :], start=(dc == 0), stop=(dc == 7))
            nc.vector.scalar_tensor_tensor(
                VPA[:, tt, :, 0:64], pv[:].rearrange("p (h w) -> p h w", h=HC),
                1.0, bv_bc[:].rearrange("p (h w) -> p h w", h=HC),
                ALU.bypass, ALU.add)
            nc.vector.tensor_copy(VPA[:, tt, :, 64:65], ones4[:].unsqueeze(2))

    # ---------------- Phase 2: attention ----------------
    with nc.named_scope("ph2"), \
         tc.tile_pool(name="ctxup", bufs=1) as ctxup, \
         tc.tile_pool(name="sps", bufs=2, space="PSUM") as sps, \
         tc.tile_pool(name="pvs", bufs=3, space="PSUM") as pvs, \
         tc.tile_pool(name="rbp", bufs=1, space="PSUM") as rbp, \
         tc.tile_pool(name="et", bufs=4) as etp, \
         tc.tile_pool(name="rcp", bufs=2) as rcpp:
        # CTXU: unnormalized acc (incl denominator row 64) staged by ACT.
        # Lives in the SBUF space freed by the ph1 xt tile.
        CTXU = ctxup.tile([65, NP, 2, QC, 512], BF16, tag="ctxu", name="ctxu")
        seq = [(p, qc, k) for p in range(NP) for qc in range(QC)
               for k in range(KT)]
        accs = {}
        ets = {}
        norm_q = []  # staggered normalize ops, one per step

        def s_step(i):
            p, qc, k = seq[i]
            qs = slice(qc * 512, (qc + 1) * 512)
            ks = slice(k * 128, (k + 1) * 128)
            st = sps.tile([128, 1024], F32, tag="st", name=f"st{p}_{qc}_{k}")
            nc.tensor.matmul(st[:, 0:512], KT_[p][0:64, ks], QT[p][0:64, qs],
                             start=True, stop=True, skip_group_check=True)
            nc.tensor.matmul(st[:, 512:1024], KT_[p][64:128, ks],
                             QT[p][64:128, qs], start=True, stop=True,
                             skip_group_check=True)
            et = etp.tile([128, 1024], BF16, tag="et", name=f"et{p}_{qc}_{k}")
            if (i % 2) == 0:
                nc.scalar.activation(et[:], st[:], AF.Exp, bias=0.0,
                                     scale=SCALE)
            else:
                nc.vector.tensor_scalar(et[:].bitcast(I16), st[:], SCH_A,
                                        SCH_B, ALU.mult, ALU.add)
            ets[i] = et

        def normalize(p, qc, accA, accB):
            """Emit the (p, qc) normalize chain as staggered closures.

            1/denominator via the single-op DVE fast reciprocal (reads the
            PSUM denominator rows directly), converted to bf16, broadcast
            across partitions with a K=1 PE outer product, and multiplied
            into CTX by DVE (one PSUM read per op)."""
            rcp16 = rcpp.tile([1, 2, 512], BF16, tag="rcp16",
                              name=f"rcp16{p}_{qc}")
            rcpb = rbp.tile([128, 512], F32, tag="rcpb", name=f"rcpb{p}_{qc}")
            RF = os.environ.get("RECIP_FAST", "0") == "1"
            if RF:
                rcpw = rcpp.tile([1, 2, 512], F32, tag="rcpw",
                                 name=f"rcpw{p}_{qc}")
                yield lambda: nc.vector.reciprocal_approx_fast(
                    rcpw[:, 0, :], accA[64:65, :])
                yield lambda: nc.vector.reciprocal_approx_fast(
                    rcpw[:, 1, :], accB[64:65, :])
                yield lambda: nc.vector.tensor_copy(rcp16[:], rcpw[:])
            else:
                def mkrecip(j, acc):
                    def f():
                        with nc.allow_low_precision(reason="bf16 recip"):
                            nc.vector.reciprocal(rcp16[:, j, :],
                                                 acc[64:65, :])
                    return f
                yield mkrecip(0, accA)
                yield mkrecip(1, accB)
            def bcasts():
                for j in range(2):
                    nc.tensor.matmul(rcpb[j * 64:(j + 1) * 64, :], ones1[:],
                                     rcp16[:, j, :], start=True, stop=True,
                                     skip_group_check=True)
            yield bcasts
            for j in range(2):
                yield lambda j=j: nc.vector.tensor_tensor(
                    CTX[j * 64:(j + 1) * 64, p, qc, :],
                    CTXU[0:64, p, j, qc, :], rcpb[j * 64:(j + 1) * 64, :],
                    ALU.mult)

        LOOKAHEAD = 2
        for i in range(LOOKAHEAD):
            s_step(i)
        for i, (p, qc, k) in enumerate(seq):
            if i + LOOKAHEAD < len(seq):
                s_step(i + LOOKAHEAD)
            if k == 0:
                accs[(p, qc)] = [
                    pvs.tile([65, 512], F32, tag="acc", name=f"acc{p}_{qc}_{j}")
                    for j in range(2)]
            accA, accB = accs[(p, qc)]
            et = ets.pop(i)
            # Each head: two 64-row matmuls accumulating into one PSUM tile
            # (row groups alternate so LDWEIGHTS hides under the prior MM and
            # each pair runs as concurrent PE row tiles).
            PV_SPLIT = os.environ.get("PV_SPLIT", "1") == "1"
            for j, acc in ((0, accA), (1, accB)):
                es = slice(j * 512, (j + 1) * 512)
                if PV_SPLIT:
                    nc.tensor.matmul(acc[:], VPA[0:64, k, 2 * p + j, :],
                                     et[0:64, es], start=(k == 0), stop=False,
                                     skip_group_check=True)
                    nc.tensor.matmul(acc[:], VPA[64:128, k, 2 * p + j, :],
                                     et[64:128, es], start=False,
                                     stop=(k == KT - 1), skip_group_check=True)
                else:
                    nc.tensor.matmul(acc[:], VPA[:, k, 2 * p + j, :],
                                     et[:, es], start=(k == 0),
                                     stop=(k == KT - 1), skip_group_check=True)
            if norm_q:
                norm_q.pop(0)()
            if k == KT - 1:
                # Stage acc (ctx^T rows + denominator row) to SBUF on ACT,
                # then stagger the normalize chain over the next steps.
                nc.scalar.copy(CTXU[:, p, 0, qc, :], accA[:])
                nc.scalar.copy(CTXU[:, p, 1, qc, :], accB[:])
                norm_q.extend(normalize(p, qc, accA, accB))
                del accs[(p, qc)]
        while norm_q:
            norm_q.pop(0)()

    # ---------------- Phase 3: Wo ----------------
    with nc.named_scope("ph3"), \
         tc.tile_pool(name="po", bufs=2, space="PSUM") as pop, \
         tc.tile_pool(name="osb", bufs=3) as osbp:
        for t in range(S // 128):
            qc, tt = divmod(t, 4)
            cs = slice(tt * 128, (tt + 1) * 128)
            po = pop.tile([128, D], F32, tag="po", name=f"po{t}")
            for n2 in range(2):
                for p in range(NP):
                    nc.tensor.matmul(
                        po[:, n2 * 512:(n2 + 1) * 512], CTX[:, p, qc, cs],
                        wo_sb[:, p, n2 * 512:(n2 + 1) * 512],
                        start=(p == 0), stop=(p == NP - 1))
            ot = osbp.tile([128, D], F32, tag="ot", name=f"ot{t}")
            nc.scalar.copy(ot[:, 0:512], po[:, 0:512])
            nc.vector.tensor_copy(ot[:, 512:1024], po[:, 512:1024])
            nc.sync.dma_start(out[t * 128:(t + 1) * 128, :], ot[:])


_CACHE = {}


def _build():
    if "nc" in _CACHE:
        return _CACHE["nc"]
    nc = bacc.Bacc("TRN2", target_bir_lowering=False, debug=False)
    ins = {
        "XT": nc.dram_tensor("XT", [D, S], BF16, kind="ExternalInput").ap(),
        "Wq": nc.dram_tensor("Wq", [D, DC], BF16, kind="ExternalInput").ap(),
        "bq": nc.dram_tensor("bq", [DC], F32, kind="ExternalInput").ap(),
        "Wk": nc.dram_tensor("Wk", [D, DC], BF16, kind="ExternalInput").ap(),
        "bk": nc.dram_tensor("bk", [DC], F32, kind="ExternalInput").ap(),
        "Wv": nc.dram_tensor("Wv", [D, DC], BF16, kind="ExternalInput").ap(),
        "bv": nc.dram_tensor("bv", [DC], F32, kind="ExternalInput").ap(),
        "Wo": nc.dram_tensor("Wo", [DC, D], BF16, kind="ExternalInput").ap(),
    }
    outp = nc.dram_tensor("out", [S, D], F32, kind="ExternalOutput").ap()
    with tile.TileContext(nc) as tcx:
        with ExitStack() as ctx:
            _emit(ctx, tcx, ins, outp)
    nc.compile()
    _CACHE["nc"] = nc
    return nc


def core_inputs(X, Wq, bq, Wk, bk, Wv, bv, Wo, core):
    b, g = divmod(core, 4)
    cs = slice(g * DC, (g + 1) * DC)
    bf = ml_dtypes.bfloat16
    return {
        "XT": np.ascontiguousarray(np.asarray(X[b]).T).astype(bf),
        "Wq": np.ascontiguousarray(Wq[:, cs]).astype(bf),
        "bq": np.ascontiguousarray(bq[cs], dtype=np.float32),
        "Wk": np.ascontiguousarray(Wk[:, cs]).astype(bf),
        "bk": np.ascontiguousarray(bk[cs], dtype=np.float32),
        "Wv": np.ascontiguousarray(Wv[:, cs]).astype(bf),
        "bv": np.ascontiguousarray(bv[cs], dtype=np.float32),
        "Wo": np.ascontiguousarray(Wo[cs, :]).astype(bf),
    }


def kernel(X, Wq, bq, Wk, bk, Wv, bv, Wo, bo, _trace=False):
    nc = _build()
    in_maps = [core_inputs(X, Wq, bq, Wk, bk, Wv, bv, Wo, c) for c in range(8)]
    res = run_bass_kernel_spmd(nc, in_maps, list(range(8)), trace=_trace)
    parts = [res.results[c]["out"] for c in range(8)]
    bo32 = np.asarray(bo, dtype=np.float32)
    full = np.stack([
        parts[0] + parts[1] + parts[2] + parts[3] + bo32,
        parts[4] + parts[5] + parts[6] + parts[7] + bo32,
    ]).astype(np.float32)
    if _trace:
        return full, res
    return full
